# revision 9
# baseline (speedup 1.0000x reference)
"""Trainium2 Bass kernel for the 10-class supervised-contrastive loss.

Problem shapes (hardcoded): preds [10, 2048, 128] f32, target [2048] int64,
log_vars [10] f32 -> scalar f32.

Sharding (8 cores, SPMD, identical program per core):
  - core c owns class c fully (16 row-strips of 128 rows of the [B,B] matrix)
  - cores 0-3 additionally own a quarter of class 8, cores 4-7 a quarter of
    class 9.  The extra class's rows are fed ROTATED (np.roll) so every core
    statically computes row-strips 0..3 of its "slot 1" class; row sums are
    permutation-invariant so rotation is safe (diagonal stays diagonal).

Host prep (O(B*D) layout/scaling only): row-normalize features, cast bf16,
transpose to G = [d, b], precompute u_pos = sum_b lab_b*ghat_b and
u_all = sum_b ghat_b.  ALL O(B^2) work runs on device.

Device, per class (slot 0 computes only the upper trapezoid cols >= a*128,
exploiting symmetry of exp(cos/T)):
  per 128-row strip a:
      C = G[:,a].T @ G[:, cols]     (bf16 matmuls, f32 PSUM, 512-col chunks)
      zero diagonal window          (DVE mul with (1-I))
      E = Exp(C/T) -> sc (bf16)     (ACT, fused accum_out row-sum)
      colsum[0, j] += ones.T @ E    (PE, ones is the 1-col STATIONARY so the
                                     load-weights cost is trivial; 512-wide
                                     moving chunks accumulate the per-column
                                     sums of all strips above the diagonal
                                     into one [1, 2048] PSUM row)
  P/R: prT = u[:, 2s:2s+2].T @ G    (2-col stationary, G moving; host
                                     transposes the [2, B] result)
Host epilogue (O(B*C)): Z[r] = rowsum[r] + colsum[r] - 1, masked mean
log-prob from P/R + analytic counts, uncertainty-weighted final sum.
"""

import ml_dtypes
import numpy as np

import concourse.bacc as bacc
import concourse.bass as bass
import concourse.mybir as mybir
import concourse.tile as tile
from concourse.bass_utils import run_bass_kernel_spmd

NUM_CLASSES = 10
B = 2048
D = 128
T = 0.07
BASE_T = 0.07
N_CORES = 8

f32 = mybir.dt.float32
bf16 = mybir.dt.bfloat16
f32r = mybir.dt.float32r
np_bf16 = ml_dtypes.bfloat16

TRACE = False
LAST_RESULT = None


def _chunks(c0, c1):
    """Split [c0, c1) at 512-aligned boundaries (PSUM bank limit)."""
    out = []
    c = c0
    while c < c1:
        nxt = min(c1, (c // 512 + 1) * 512)
        out.append((c, nxt))
        c = nxt
    return out


def _build_nc():
    nc = bacc.Bacc(None, target_bir_lowering=False)

    g_dram = [
        nc.dram_tensor(f"g{s}", [128, B], bf16, kind="ExternalInput")
        for s in range(2)
    ]
    u_dram = nc.dram_tensor("u", [128, 4], bf16, kind="ExternalInput")
    ones_dram = nc.dram_tensor("ones1", [128, 1], f32r, kind="ExternalInput")
    masknd_dram = nc.dram_tensor("masknd", [128, 128], f32, kind="ExternalInput")
    rows_dram = nc.dram_tensor("rows", [128, 40], f32, kind="ExternalOutput")
    colsum_dram = nc.dram_tensor("colsum", [1, B - 128], f32, kind="ExternalOutput")
    prt_dram = nc.dram_tensor("prt", [2, B + 512], f32, kind="ExternalOutput")

    EXP = mybir.ActivationFunctionType.Exp

    with tile.TileContext(nc) as tc:
        with (
            tc.tile_pool(name="const", bufs=1) as constp,
            tc.tile_pool(name="gmat", bufs=1) as gmatp,
            tc.tile_pool(name="scp", bufs=4) as scp,
        ):
            # DMA issue order matters: each dma_start costs ~0.65us of issue
            # time on the sync sequencer, and the first cos matmul gates on
            # g0.  Load g0 first (split so compute can start after the first
            # half), small constants next, g1 (not needed until slot 1) last.
            G = []
            for s in range(2):
                G.append(gmatp.tile([128, B], bf16, tag=f"G{s}", name=f"G{s}"))
            for k in range(4):
                nc.sync.dma_start(
                    G[0][:, 512 * k : 512 * (k + 1)],
                    g_dram[0][:, 512 * k : 512 * (k + 1)],
                )
            masknd_sb = constp.tile([128, 128], f32, tag="masknd")
            nc.sync.dma_start(masknd_sb[:], masknd_dram[:])
            ones_sb = constp.tile([128, 1], f32r, tag="ones1")
            nc.sync.dma_start(ones_sb[:], ones_dram[:])
            u_sb = constp.tile([128, 4], bf16, tag="u")
            nc.sync.dma_start(u_sb[:], u_dram[:])
            nc.sync.dma_start(G[1][:], g_dram[1][:])
            rows_sb = constp.tile([128, 40], f32, tag="rows")
            nc.scalar.memzero(rows_sb[:])
            colsum_sb = constp.tile([1, B - 128], f32, tag="colsum")
            prt_sb = constp.tile([2, B + 512], f32, tag="prt")

            def emit_strip(cpp, s, rb, u):
                """Cos matmuls + diag mask + Exp for one 128-row strip.

                Returns the bf16 exp tiles [(base, lo, sc_tile), ...] for the
                colsum pass (slot 0).
                """
                lhsT = G[s][:, bass.ts(rb, 128)]
                if s == 0:
                    halves = (
                        [(0, rb * 128, 1024), (1024, 1024, 2048)]
                        if rb < 8
                        else [(1024, rb * 128, 2048)]
                    )
                else:
                    halves = [(0, 0, 1024), (1024, 1024, 2048)]
                out_tiles = []
                for hi, (base, c0, c1) in enumerate(halves):
                    cp = cpp.tile([128, 1024], f32, tag="cp", name=f"cp{u}_{hi}")
                    has_diag = c0 <= rb * 128 < c1
                    for ci, (a0, a1) in enumerate(_chunks(c0, c1)):
                        nc.tensor.matmul(
                            cp[:, a0 - base : a1 - base],
                            lhsT,
                            G[s][:, a0:a1],
                            start=True,
                            stop=True,
                        )
                        if ci == 0 and has_diag:
                            # Mask as soon as the diag chunk lands so the
                            # DVE->ACT chain starts before the tail chunks.
                            w0 = rb * 128 - base
                            nc.vector.tensor_mul(
                                cp[:, w0 : w0 + 128],
                                cp[:, w0 : w0 + 128],
                                masknd_sb[:],
                            )
                    sc = scp.tile([128, 1024], f32r, tag="sc", name=f"sc{u}_{hi}")
                    nc.scalar.activation(
                        sc[:, c0 - base : c1 - base],
                        cp[:, c0 - base : c1 - base],
                        EXP,
                        scale=1.0 / T,
                        accum_out=rows_sb[:, 2 * u + hi : 2 * u + hi + 1],
                    )
                    out_tiles.append((base, c0, sc))
                return out_tiles

            with tc.tile_pool(name="cpsum", bufs=2, space="PSUM") as cpp:
                # ---- slot 0: own class, upper trapezoid + colsum mirror ----
                with tc.tile_pool(name="cspsum", bufs=1, space="PSUM") as csp:
                    cs = csp.tile([1, B], f32, tag="cs", name="cs")

                    def emit_colsum(rb, tiles):
                        # Per-column sums of all tiles strictly above the
                        # diagonal: strip rb contributes cols >= (rb+1)*128.
                        for base, c0, sc in tiles:
                            d0 = max((rb + 1) * 128, base)
                            for a0, a1 in _chunks(d0, base + 1024):
                                nc.tensor.matmul(
                                    cs[0:1, a0:a1],
                                    ones_sb[:],
                                    sc[:, a0 - base : a1 - base],
                                    start=(rb == 0),
                                    stop=(a1 // 128 - 2 == rb),
                                    skip_group_check=True,
                                )

                    # One-strip lag: emit strip rb's colsum matmuls after
                    # strip rb+1's cos/Exp so the PE never stalls waiting for
                    # the ACT engine's exp output of the strip it just built.
                    pending = None
                    for rb in range(16):
                        tiles = emit_strip(cpp, 0, rb, rb)
                        if pending is not None:
                            emit_colsum(rb - 1, pending)
                        pending = tiles
                    emit_colsum(15, pending)

                    # ---- slot 1: quarter of the shared class, full rows ---
                    # P/R chunks: (slot, col0, dest offset in prt_sb)
                    pr_jobs = [(0, k * 512, k * 512) for k in range(4)] + [
                        (1, 0, 2048)
                    ]

                    def emit_pr(job):
                        # pp tiles rotate through the cp pool's buffers.
                        s, a0, o0 = job
                        pp = cpp.tile([2, 512], f32, tag="cp", name=f"pp{o0}")
                        nc.tensor.matmul(
                            pp[:],
                            u_sb[:, 2 * s : 2 * s + 2],
                            G[s][:, a0 : a0 + 512],
                            start=True,
                            stop=True,
                        )
                        nc.vector.tensor_copy(prt_sb[:, o0 : o0 + 512], pp[:])

                    cs_cuts = [128, 608, 1088, 1568, B]
                    for rb in range(4):
                        emit_strip(cpp, 1, rb, 16 + rb)
                        d0, d1 = cs_cuts[rb], cs_cuts[rb + 1]
                        nc.vector.tensor_copy(
                            colsum_sb[0:1, d0 - 128 : d1 - 128], cs[0:1, d0:d1]
                        )
                    while pr_jobs:
                        emit_pr(pr_jobs.pop(0))

            nc.sync.dma_start(prt_dram[:], prt_sb[:])
            nc.sync.dma_start(colsum_dram[:], colsum_sb[:])
            nc.sync.dma_start(rows_dram[:], rows_sb[:])
    nc.finalize()
    return nc


_NC_CACHE = None


def _get_nc():
    global _NC_CACHE
    if _NC_CACHE is None:
        _NC_CACHE = _build_nc()
    return _NC_CACHE


def kernel(preds, target, log_vars):
    global LAST_RESULT
    preds = np.asarray(preds, dtype=np.float32)
    target = np.asarray(target)
    log_vars = np.asarray(log_vars, dtype=np.float32)

    onehot = (target[None, :] == np.arange(NUM_CLASSES, dtype=target.dtype)[:, None])
    onehot = onehot.astype(np.float32)  # [10, B]
    npos = onehot.sum(axis=1).astype(np.float64)  # [10]

    # Host prep: row-normalize (f32 stats), cast bf16, transpose.
    norms = np.sqrt((preds.astype(np.float32) ** 2).sum(axis=2, dtype=np.float32))
    ghat = (preds / norms[:, :, None]).astype(np_bf16)  # [10, B, D]
    ghf32 = ghat.astype(np.float32)

    masknd = np.ascontiguousarray(1.0 - np.eye(128, dtype=np.float32))
    ones1 = np.ones((128, 1), dtype=np.float32)

    in_maps = []
    for c in range(N_CORES):
        cls1 = 8 + c // 4
        off = 512 * (c % 4)
        im = {"masknd": masknd, "ones1": ones1}
        u_np = np.zeros((128, 4), dtype=np.float32)
        for s, (cls, o) in enumerate([(c, 0), (cls1, off)]):
            gh = np.roll(ghat[cls], -o, axis=0) if o else ghat[cls]
            im[f"g{s}"] = np.ascontiguousarray(gh.T)  # [128, 2048] [d, b]
            u_np[:, 2 * s] = (onehot[cls][:, None] * ghf32[cls]).sum(axis=0)
            u_np[:, 2 * s + 1] = ghf32[cls].sum(axis=0)
        im["u"] = u_np.astype(np_bf16)
        in_maps.append(im)

    nc = _get_nc()
    res = run_bass_kernel_spmd(nc, in_maps, list(range(N_CORES)), trace=TRACE)
    LAST_RESULT = res

    # Reassemble per-(class,row) stats.
    Z = np.zeros((NUM_CLASSES, B), dtype=np.float64)
    P = np.zeros((NUM_CLASSES, B), dtype=np.float64)
    R = np.zeros((NUM_CLASSES, B), dtype=np.float64)
    idx = np.arange(128)
    for c in range(N_CORES):
        rows = np.asarray(res.results[c]["rows"], dtype=np.float64)  # [128, 40]
        colsum = np.asarray(res.results[c]["colsum"], dtype=np.float64)[0]  # [1920]
        prt = np.asarray(res.results[c]["prt"], dtype=np.float64)  # [2, 2560]
        cls1 = 8 + c // 4
        off = 512 * (c % 4)
        for rb in range(16):  # slot 0: class c
            tot = rows[:, 2 * rb].copy()
            if rb < 8:
                tot += rows[:, 2 * rb + 1]
            r = rb * 128 + idx
            if rb >= 1:
                tot += colsum[r - 128]
            Z[c, r] = tot - 1.0
        P[c, :] = prt[0, 0:B]
        R[c, :] = prt[1, 0:B]
        for rb in range(4):  # slot 1: quarter of cls1, rotated by -off
            u = 16 + rb
            r = (off + rb * 128 + idx) % B
            Z[cls1, r] = rows[:, 2 * u] + rows[:, 2 * u + 1] - 1.0
        j = np.arange(512)
        r1 = (off + j) % B
        P[cls1, r1] = prt[0, B + j]
        R[cls1, r1] = prt[1, B + j]

    lab = onehot.astype(np.float64)
    masked_cos = lab * P + (1.0 - lab) * (R - P)
    masked_logits_sum = (masked_cos - 1.0) / T
    cnt = lab * npos[:, None] + (1.0 - lab) * (B - npos[:, None]) - 1.0
    mlpp = masked_logits_sum / cnt - np.log(Z)
    losses = -(T / BASE_T) * mlpp.mean(axis=1)  # [10]
    lv = log_vars.astype(np.float64)
    final = np.sum(np.exp(-lv) * losses + lv)
    return np.float32(final)


# revision 10
# speedup vs baseline: 1.0101x; 1.0101x over previous
"""Trainium2 Bass kernel for the 10-class supervised-contrastive loss.

Problem shapes (hardcoded): preds [10, 2048, 128] f32, target [2048] int64,
log_vars [10] f32 -> scalar f32.

Sharding (8 cores, SPMD, identical program per core):
  - core c owns class c fully (16 row-strips of 128 rows of the [B,B] matrix)
  - cores 0-3 additionally own a quarter of class 8, cores 4-7 a quarter of
    class 9.  The extra class's rows are fed ROTATED (np.roll) so every core
    statically computes row-strips 0..3 of its "slot 1" class; row sums are
    permutation-invariant so rotation is safe (diagonal stays diagonal).

Host prep (O(B*D) layout/scaling only): row-normalize features, cast bf16,
transpose to G = [d, b], precompute u_pos = sum_b lab_b*ghat_b and
u_all = sum_b ghat_b.  ALL O(B^2) work runs on device.

Device, per class (slot 0 computes only the upper trapezoid cols >= a*128,
exploiting symmetry of exp(cos/T)):
  per 128-row strip a:
      C = G[:,a].T @ G[:, cols]     (bf16 matmuls, f32 PSUM, 512-col chunks)
      zero diagonal window          (DVE mul with (1-I))
      E = Exp(C/T) -> sc (bf16)     (ACT, fused accum_out row-sum)
      colsum[0, j] += ones.T @ E    (PE, ones is the 1-col STATIONARY so the
                                     load-weights cost is trivial; 512-wide
                                     moving chunks accumulate the per-column
                                     sums of all strips above the diagonal
                                     into one [1, 2048] PSUM row)
  P/R: prT = u[:, 2s:2s+2].T @ G    (2-col stationary, G moving; host
                                     transposes the [2, B] result)
Host epilogue (O(B*C)): Z[r] = rowsum[r] + colsum[r] - 1, masked mean
log-prob from P/R + analytic counts, uncertainty-weighted final sum.
"""

import ml_dtypes
import numpy as np

import concourse.bacc as bacc
import concourse.bass as bass
import concourse.mybir as mybir
import concourse.tile as tile
from concourse.bass_utils import run_bass_kernel_spmd

NUM_CLASSES = 10
B = 2048
D = 128
T = 0.07
BASE_T = 0.07
N_CORES = 8

f32 = mybir.dt.float32
bf16 = mybir.dt.bfloat16
f32r = mybir.dt.float32r
np_bf16 = ml_dtypes.bfloat16

TRACE = False
LAST_RESULT = None


def _chunks(c0, c1):
    """Split [c0, c1) at 512-aligned boundaries (PSUM bank limit)."""
    out = []
    c = c0
    while c < c1:
        nxt = min(c1, (c // 512 + 1) * 512)
        out.append((c, nxt))
        c = nxt
    return out


def _build_nc():
    nc = bacc.Bacc(None, target_bir_lowering=False)

    g_dram = [
        nc.dram_tensor(f"g{s}", [128, B], bf16, kind="ExternalInput")
        for s in range(2)
    ]
    u_dram = nc.dram_tensor("u", [128, 4], bf16, kind="ExternalInput")
    ones_dram = nc.dram_tensor("ones1", [128, 1], bf16, kind="ExternalInput")
    masknd_dram = nc.dram_tensor("masknd", [128, 128], f32, kind="ExternalInput")
    rows_dram = nc.dram_tensor("rows", [128, 40], f32, kind="ExternalOutput")
    colsum_dram = nc.dram_tensor("colsum", [1, B - 128], f32, kind="ExternalOutput")
    prt_dram = nc.dram_tensor("prt", [2, B + 512], f32, kind="ExternalOutput")

    EXP = mybir.ActivationFunctionType.Exp

    with tile.TileContext(nc) as tc:
        with (
            tc.tile_pool(name="const", bufs=1) as constp,
            tc.tile_pool(name="gmat", bufs=1) as gmatp,
            tc.tile_pool(name="scp", bufs=4) as scp,
        ):
            # DMA issue order matters: each dma_start costs ~0.65us of issue
            # time on the sync sequencer, and the first cos matmul gates on
            # g0.  Load g0 first (split so compute can start after the first
            # half), small constants next, g1 (not needed until slot 1) last.
            G = []
            for s in range(2):
                G.append(gmatp.tile([128, B], bf16, tag=f"G{s}", name=f"G{s}"))
            for k in range(4):
                nc.sync.dma_start(
                    G[0][:, 512 * k : 512 * (k + 1)],
                    g_dram[0][:, 512 * k : 512 * (k + 1)],
                )
            masknd_sb = constp.tile([128, 128], f32, tag="masknd")
            nc.sync.dma_start(masknd_sb[:], masknd_dram[:])
            ones_sb = constp.tile([128, 1], bf16, tag="ones1")
            nc.sync.dma_start(ones_sb[:], ones_dram[:])
            u_sb = constp.tile([128, 4], bf16, tag="u")
            nc.sync.dma_start(u_sb[:], u_dram[:])
            nc.sync.dma_start(G[1][:], g_dram[1][:])
            rows_sb = constp.tile([128, 40], f32, tag="rows")
            nc.scalar.memzero(rows_sb[:])
            colsum_sb = constp.tile([1, B - 128], f32, tag="colsum")
            prt_sb = constp.tile([2, B + 512], f32, tag="prt")

            def emit_strip(cpp, s, rb, u):
                """Cos matmuls + diag mask + Exp for one 128-row strip.

                Returns the bf16 exp tiles [(base, lo, sc_tile), ...] for the
                colsum pass (slot 0).
                """
                lhsT = G[s][:, bass.ts(rb, 128)]
                if s == 0:
                    halves = (
                        [(0, rb * 128, 1024), (1024, 1024, 2048)]
                        if rb < 8
                        else [(1024, rb * 128, 2048)]
                    )
                else:
                    halves = [(0, 0, 1024), (1024, 1024, 2048)]
                out_tiles = []
                for hi, (base, c0, c1) in enumerate(halves):
                    cp = cpp.tile([128, 1024], f32, tag="cp", name=f"cp{u}_{hi}")
                    has_diag = c0 <= rb * 128 < c1
                    for ci, (a0, a1) in enumerate(_chunks(c0, c1)):
                        nc.tensor.matmul(
                            cp[:, a0 - base : a1 - base],
                            lhsT,
                            G[s][:, a0:a1],
                            start=True,
                            stop=True,
                        )
                        if ci == 0 and has_diag:
                            # Mask as soon as the diag chunk lands so the
                            # DVE->ACT chain starts before the tail chunks.
                            w0 = rb * 128 - base
                            nc.vector.tensor_mul(
                                cp[:, w0 : w0 + 128],
                                cp[:, w0 : w0 + 128],
                                masknd_sb[:],
                            )
                    sc = scp.tile([128, 1024], bf16, tag="sc", name=f"sc{u}_{hi}")
                    nc.scalar.activation(
                        sc[:, c0 - base : c1 - base],
                        cp[:, c0 - base : c1 - base],
                        EXP,
                        scale=1.0 / T,
                        accum_out=rows_sb[:, 2 * u + hi : 2 * u + hi + 1],
                    )
                    out_tiles.append((base, c0, sc))
                return out_tiles

            with tc.tile_pool(name="cpsum", bufs=2, space="PSUM") as cpp:
                # ---- slot 0: own class, upper trapezoid + colsum mirror ----
                with tc.tile_pool(name="cspsum", bufs=1, space="PSUM") as csp:
                    cs = csp.tile([1, B], f32, tag="cs", name="cs")

                    def emit_colsum(rb, tiles):
                        # Per-column sums of all tiles strictly above the
                        # diagonal: strip rb contributes cols >= (rb+1)*128.
                        for base, c0, sc in tiles:
                            d0 = max((rb + 1) * 128, base)
                            for a0, a1 in _chunks(d0, base + 1024):
                                nc.tensor.matmul(
                                    cs[0:1, a0:a1],
                                    ones_sb[:],
                                    sc[:, a0 - base : a1 - base],
                                    start=(rb == 0),
                                    stop=(a1 // 128 - 2 == rb),
                                    skip_group_check=True,
                                )

                    # One-strip lag: emit strip rb's colsum matmuls after
                    # strip rb+1's cos/Exp so the PE never stalls waiting for
                    # the ACT engine's exp output of the strip it just built.
                    pending = None
                    for rb in range(16):
                        tiles = emit_strip(cpp, 0, rb, rb)
                        if pending is not None:
                            emit_colsum(rb - 1, pending)
                        pending = tiles
                    emit_colsum(15, pending)

                    # ---- slot 1: quarter of the shared class, full rows ---
                    # P/R chunks: (slot, col0, dest offset in prt_sb)
                    pr_jobs = [(0, k * 512, k * 512) for k in range(4)] + [
                        (1, 0, 2048)
                    ]

                    def emit_pr(job):
                        # pp tiles rotate through the cp pool's buffers.
                        s, a0, o0 = job
                        pp = cpp.tile([2, 512], f32, tag="cp", name=f"pp{o0}")
                        nc.tensor.matmul(
                            pp[:],
                            u_sb[:, 2 * s : 2 * s + 2],
                            G[s][:, a0 : a0 + 512],
                            start=True,
                            stop=True,
                        )
                        nc.vector.tensor_copy(prt_sb[:, o0 : o0 + 512], pp[:])

                    cs_cuts = [128, 608, 1088, 1568, B]
                    for rb in range(4):
                        emit_strip(cpp, 1, rb, 16 + rb)
                        d0, d1 = cs_cuts[rb], cs_cuts[rb + 1]
                        nc.vector.tensor_copy(
                            colsum_sb[0:1, d0 - 128 : d1 - 128], cs[0:1, d0:d1]
                        )
                    while pr_jobs:
                        emit_pr(pr_jobs.pop(0))

            nc.sync.dma_start(prt_dram[:], prt_sb[:])
            nc.sync.dma_start(colsum_dram[:], colsum_sb[:])
            nc.sync.dma_start(rows_dram[:], rows_sb[:])
    nc.finalize()
    return nc


_NC_CACHE = None


def _get_nc():
    global _NC_CACHE
    if _NC_CACHE is None:
        _NC_CACHE = _build_nc()
    return _NC_CACHE


def kernel(preds, target, log_vars):
    global LAST_RESULT
    preds = np.asarray(preds, dtype=np.float32)
    target = np.asarray(target)
    log_vars = np.asarray(log_vars, dtype=np.float32)

    onehot = (target[None, :] == np.arange(NUM_CLASSES, dtype=target.dtype)[:, None])
    onehot = onehot.astype(np.float32)  # [10, B]
    npos = onehot.sum(axis=1).astype(np.float64)  # [10]

    # Host prep: row-normalize (f32 stats), cast bf16, transpose.
    norms = np.sqrt((preds.astype(np.float32) ** 2).sum(axis=2, dtype=np.float32))
    ghat = (preds / norms[:, :, None]).astype(np_bf16)  # [10, B, D]
    ghf32 = ghat.astype(np.float32)

    masknd = np.ascontiguousarray(1.0 - np.eye(128, dtype=np.float32))
    ones1 = np.ones((128, 1), dtype=np_bf16)

    in_maps = []
    for c in range(N_CORES):
        cls1 = 8 + c // 4
        off = 512 * (c % 4)
        im = {"masknd": masknd, "ones1": ones1}
        u_np = np.zeros((128, 4), dtype=np.float32)
        for s, (cls, o) in enumerate([(c, 0), (cls1, off)]):
            gh = np.roll(ghat[cls], -o, axis=0) if o else ghat[cls]
            im[f"g{s}"] = np.ascontiguousarray(gh.T)  # [128, 2048] [d, b]
            u_np[:, 2 * s] = (onehot[cls][:, None] * ghf32[cls]).sum(axis=0)
            u_np[:, 2 * s + 1] = ghf32[cls].sum(axis=0)
        im["u"] = u_np.astype(np_bf16)
        in_maps.append(im)

    nc = _get_nc()
    res = run_bass_kernel_spmd(nc, in_maps, list(range(N_CORES)), trace=TRACE)
    LAST_RESULT = res

    # Reassemble per-(class,row) stats.
    Z = np.zeros((NUM_CLASSES, B), dtype=np.float64)
    P = np.zeros((NUM_CLASSES, B), dtype=np.float64)
    R = np.zeros((NUM_CLASSES, B), dtype=np.float64)
    idx = np.arange(128)
    for c in range(N_CORES):
        rows = np.asarray(res.results[c]["rows"], dtype=np.float64)  # [128, 40]
        colsum = np.asarray(res.results[c]["colsum"], dtype=np.float64)[0]  # [1920]
        prt = np.asarray(res.results[c]["prt"], dtype=np.float64)  # [2, 2560]
        cls1 = 8 + c // 4
        off = 512 * (c % 4)
        for rb in range(16):  # slot 0: class c
            tot = rows[:, 2 * rb].copy()
            if rb < 8:
                tot += rows[:, 2 * rb + 1]
            r = rb * 128 + idx
            if rb >= 1:
                tot += colsum[r - 128]
            Z[c, r] = tot - 1.0
        P[c, :] = prt[0, 0:B]
        R[c, :] = prt[1, 0:B]
        for rb in range(4):  # slot 1: quarter of cls1, rotated by -off
            u = 16 + rb
            r = (off + rb * 128 + idx) % B
            Z[cls1, r] = rows[:, 2 * u] + rows[:, 2 * u + 1] - 1.0
        j = np.arange(512)
        r1 = (off + j) % B
        P[cls1, r1] = prt[0, B + j]
        R[cls1, r1] = prt[1, B + j]

    lab = onehot.astype(np.float64)
    masked_cos = lab * P + (1.0 - lab) * (R - P)
    masked_logits_sum = (masked_cos - 1.0) / T
    cnt = lab * npos[:, None] + (1.0 - lab) * (B - npos[:, None]) - 1.0
    mlpp = masked_logits_sum / cnt - np.log(Z)
    losses = -(T / BASE_T) * mlpp.mean(axis=1)  # [10]
    lv = log_vars.astype(np.float64)
    final = np.sum(np.exp(-lv) * losses + lv)
    return np.float32(final)
